# revision 1
# baseline (speedup 1.0000x reference)
"""Trainium2 Bass kernel for a dense transformer block (RMSNorm + GQA attention
with RoPE + SwiGLU MLP), distributed over 8 NeuronCores.

Sharding: data-parallel over (batch, query-block). Core c handles batch c//4,
queries [512*(c%4), 512*(c%4+1)). Each core computes K/V for all 2048 keys of
its batch (communication-free); causality is applied via per-core mask data so
the SPMD program is identical on every core.

Device tensors live in transposed layout [feature, token] so contractions sit
on the partition axis. Weights are host-packed into lhsT tile layout. Softmax
runs without max-subtraction (scores have sigma~0.8; exp cannot overflow),
letting attention numerators and denominators accumulate directly in PSUM.

The causal mask is a single [128, 2432] "staircase": the mask tile for key
subtile ks is its slice at offset (15-ks)*128, so one small tensor serves all
16 subtiles and the slice offsets are core-independent.
"""

import sys

sys.path.insert(0, "/opt/trn_rl_repo")

import numpy as np

B, S, D = 2, 2048, 2048
H, KVH, HD = 16, 8, 128
FF = 5504
P = 128
DS = D // P          # 16 subtiles of D
FFC = FF // P        # 43 subtiles of FF
QN = 512             # queries per core
KC = S // 512        # 4 key chunks
NKS = S // P         # 16 key subtiles
MEXT = S + 512 - P   # 2432 staircase width
EPS = 1e-5
NCORES = 8
F_GROUPS = ((0, 11), (11, 22), (22, 33), (33, FFC))

_prog = None


def _build():
    from contextlib import ExitStack

    import concourse.bass as bass  # noqa: F401
    import concourse.tile as tile
    from concourse import bacc, mybir
    from concourse.masks import make_identity

    f32 = mybir.dt.float32
    AF = mybir.ActivationFunctionType
    OP = mybir.AluOpType

    nc = bacc.Bacc("TRN2", target_bir_lowering=False, debug=False)

    xT = nc.dram_tensor("xT", [D, S], f32, kind="ExternalInput").ap()
    xTq = nc.dram_tensor("xTq", [D, QN], f32, kind="ExternalInput").ap()
    wq = nc.dram_tensor("wq_pk", [H, P, DS, P], f32, kind="ExternalInput").ap()
    wk = nc.dram_tensor("wk_pk", [KVH, P, DS, P], f32, kind="ExternalInput").ap()
    wv = nc.dram_tensor("wv_pk", [KVH, P, DS, P], f32, kind="ExternalInput").ap()
    wo = nc.dram_tensor("wo_pk", [DS, P, H, P], f32, kind="ExternalInput").ap()
    wg = nc.dram_tensor("wg_pk", [FFC, P, DS, P], f32, kind="ExternalInput").ap()
    wu = nc.dram_tensor("wu_pk", [FFC, P, DS, P], f32, kind="ExternalInput").ap()
    wd = nc.dram_tensor("wd_pk", [DS, P, FFC, P], f32, kind="ExternalInput").ap()
    cosk = nc.dram_tensor("cos_k", [P, S], f32, kind="ExternalInput").ap()
    sink = nc.dram_tensor("sin_k", [P, S], f32, kind="ExternalInput").ap()
    cosq = nc.dram_tensor("cos_q", [P, QN], f32, kind="ExternalInput").ap()
    sinq = nc.dram_tensor("sin_q", [P, QN], f32, kind="ExternalInput").ap()
    mask = nc.dram_tensor("mask_ext", [P, MEXT], f32, kind="ExternalInput").ap()
    out_rows = nc.dram_tensor("out_rows", [QN, D], f32, kind="ExternalOutput").ap()

    k_spill = nc.dram_tensor("k_spill", [KVH, P, S], f32).ap()
    v_spill = nc.dram_tensor("v_spill", [NKS, P, KVH * P], f32).ap()

    xT_r = xT.rearrange("(ds p) t -> p ds t", p=P)
    xTq_r = xTq.rearrange("(ds p) t -> p ds t", p=P)
    v_spill_r = v_spill.rearrange("kb p n -> p kb n")

    with tile.TileContext(nc) as tc, ExitStack() as ctx:
        # Tag-grouped pools; static SBUF budget/partition ~201KB of 208KB.
        const_pool = ctx.enter_context(tc.tile_pool(name="const", bufs=1))   # ~1.2KB
        big_pool = ctx.enter_context(tc.tile_pool(name="big", bufs=2))       # 64KB
        attn_pool = ctx.enter_context(tc.tile_pool(name="attn", bufs=1))     # 32KB
        mask_pool = ctx.enter_context(tc.tile_pool(name="mask", bufs=1))     # 9.5KB
        hid_pool = ctx.enter_context(tc.tile_pool(name="hid", bufs=1))       # 22KB
        w_pool = ctx.enter_context(tc.tile_pool(name="w", bufs=3))           # 24KB
        kh_pool = ctx.enter_context(tc.tile_pool(name="kh", bufs=1))         # 8KB
        vh_pool = ctx.enter_context(tc.tile_pool(name="vh", bufs=2))         # 16KB
        stage_pool = ctx.enter_context(tc.tile_pool(name="stage", bufs=3))   # 6KB
        sq_pool = ctx.enter_context(tc.tile_pool(name="sq", bufs=2))         # 4KB
        small_pool = ctx.enter_context(tc.tile_pool(name="small", bufs=3))   # 6KB
        rope_pool = ctx.enter_context(tc.tile_pool(name="rope", bufs=2))     # 4KB
        ropec_pool = ctx.enter_context(tc.tile_pool(name="ropec", bufs=3))   # 6KB
        ex_pool = ctx.enter_context(tc.tile_pool(name="ex", bufs=2))         # 4KB
        psum = ctx.enter_context(tc.tile_pool(name="ps", bufs=2, space="PSUM"))

        ones_t = const_pool.tile([P, P], f32, tag="ones")
        nc.vector.memset(ones_t, 1.0)
        ident = const_pool.tile([P, P], f32, tag="ident")
        make_identity(nc, ident)
        eps_t = const_pool.tile([P, 1], f32, tag="eps")
        nc.vector.memset(eps_t, EPS)
        mask_t = mask_pool.tile([P, MEXT], f32, tag="mask")
        nc.sync.dma_start(mask_t, mask)

        def rmsnorm(xt, dst, ncols):
            """dst[:, i, :] = normalized xt[:, i, :]; xt/dst may be the same tile."""
            ps_ss = psum.tile([P, ncols], f32, tag="proj")
            for i in range(DS):
                sq = sq_pool.tile([P, ncols], f32, tag="sq")
                nc.vector.tensor_tensor(sq, xt[:, i, :], xt[:, i, :], OP.mult)
                nc.tensor.matmul(
                    ps_ss, lhsT=ones_t, rhs=sq, start=(i == 0), stop=(i == DS - 1)
                )
            sqv = small_pool.tile([P, ncols], f32, tag="small")
            nc.scalar.activation(sqv, ps_ss, AF.Sqrt, bias=eps_t, scale=1.0 / D)
            rstd = small_pool.tile([P, ncols], f32, tag="small")
            nc.vector.reciprocal(rstd, sqv)
            for i in range(DS):
                nc.vector.tensor_tensor(dst[:, i, :], xt[:, i, :], rstd, OP.mult)

        def rope(ps_in, cos_ap, sin_ap, out_ap):
            """out = ps_in * cos + rotate_half(ps_in) * sin  (sin pre-signed)."""
            a = rope_pool.tile([P, QN], f32, tag="rope")
            nc.vector.tensor_tensor(a, ps_in, cos_ap, OP.mult)
            b = rope_pool.tile([P, QN], f32, tag="rope")
            nc.vector.tensor_tensor(b[0:64, :], ps_in[64:128, :], sin_ap[0:64, :], OP.mult)
            nc.vector.tensor_tensor(b[64:128, :], ps_in[0:64, :], sin_ap[64:128, :], OP.mult)
            nc.vector.tensor_tensor(out_ap, a, b, OP.add)

        # ---------- Phase A: K/V projections over all keys (chunks of 512) ----
        for kc in range(KC):
            ksl = slice(kc * 512, (kc + 1) * 512)
            xt = big_pool.tile([P, DS, 512], f32, tag="big")
            nc.sync.dma_start(xt, xT_r[:, :, ksl])
            rmsnorm(xt, xt, 512)
            cosk_t = ropec_pool.tile([P, 512], f32, tag="ropec")
            nc.sync.dma_start(cosk_t, cosk[:, ksl])
            sink_t = ropec_pool.tile([P, 512], f32, tag="ropec")
            nc.sync.dma_start(sink_t, sink[:, ksl])

            # K projection + RoPE + spill, one 128-dim chunk per KV head
            for kvh in range(KVH):
                wkt = w_pool.tile([P, DS, P], f32, tag="w")
                nc.sync.dma_start(wkt, wk[kvh])
                ps_k = psum.tile([P, 512], f32, tag="score")
                for i in range(DS):
                    nc.tensor.matmul(
                        ps_k, lhsT=wkt[:, i, :], rhs=xt[:, i, :],
                        start=(i == 0), stop=(i == DS - 1),
                    )
                kst = stage_pool.tile([P, 512], f32, tag="stage")
                rope(ps_k, cosk_t, sink_t, kst)
                nc.sync.dma_start(k_spill[kvh][:, ksl], kst)

            # V projection (vT chunks), PE-transpose to [key, dim], spill
            for kvh in range(KVH):
                wvt = w_pool.tile([P, DS, P], f32, tag="w")
                nc.sync.dma_start(wvt, wv[kvh])
                ps_vt = psum.tile([P, 512], f32, tag="att")
                for i in range(DS):
                    nc.tensor.matmul(
                        ps_vt, lhsT=wvt[:, i, :], rhs=xt[:, i, :],
                        start=(i == 0), stop=(i == DS - 1),
                    )
                vts = stage_pool.tile([P, 512], f32, tag="stage")
                nc.scalar.copy(vts, ps_vt)
                for t in range(4):
                    ps_tr = psum.tile([P, P], f32, tag="den")
                    nc.tensor.transpose(ps_tr, vts[:, t * P : (t + 1) * P], ident)
                    trs = stage_pool.tile([P, P], f32, tag="stage")
                    nc.vector.tensor_copy(out=trs, in_=ps_tr)
                    kb = kc * 4 + t
                    nc.sync.dma_start(v_spill[kb][:, kvh * P : (kvh + 1) * P], trs)

        # ---------- Phase A': Q projection + RoPE (own 512 queries) ----------
        xtq = big_pool.tile([P, DS, QN], f32, tag="big")
        nc.sync.dma_start(xtq, xTq_r)
        rmsnorm(xtq, xtq, QN)
        cosq_t = ropec_pool.tile([P, QN], f32, tag="ropec")
        nc.sync.dma_start(cosq_t, cosq)
        sinq_t = ropec_pool.tile([P, QN], f32, tag="ropec")
        nc.sync.dma_start(sinq_t, sinq)
        qrotT = big_pool.tile([P, H, QN], f32, tag="big")
        for h in range(H):
            wqt = w_pool.tile([P, DS, P], f32, tag="w")
            nc.sync.dma_start(wqt, wq[h])
            ps_q = psum.tile([P, QN], f32, tag="score")
            for i in range(DS):
                nc.tensor.matmul(
                    ps_q, lhsT=wqt[:, i, :], rhs=xtq[:, i, :],
                    start=(i == 0), stop=(i == DS - 1),
                )
            rope(ps_q, cosq_t, sinq_t, qrotT[:, h, :])

        # ---------- Phase B: attention ---------------------------------------
        attn_outT = attn_pool.tile([P, H, QN], f32, tag="attn_out")
        kh = None
        vh = None
        for h in range(H):
            kvh = h // 2
            if h % 2 == 0:
                kh = kh_pool.tile([P, S], f32, tag="kh")
                nc.sync.dma_start(kh, k_spill[kvh])
                vh = vh_pool.tile([P, NKS, P], f32, tag="vh")
                nc.sync.dma_start(vh, v_spill_r[:, :, kvh * P : (kvh + 1) * P])
            ps_att = psum.tile([P, QN], f32, tag="att")
            # exp tiles accumulate on DVE (PE has no slack; DVE does), with a
            # single ones-matmul per head for the cross-partition denominator.
            den_acc = stage_pool.tile([P, QN], f32, tag="stage")
            for ks in range(NKS):
                ps_s = psum.tile([P, QN], f32, tag="score")
                nc.tensor.matmul(
                    ps_s, lhsT=kh[:, ks * P : (ks + 1) * P], rhs=qrotT[:, h, :],
                    start=True, stop=True,
                )
                ex = ex_pool.tile([P, QN], f32, tag="ex")
                nc.scalar.activation(ex, ps_s, AF.Exp)
                j0 = (NKS - 1 - ks) * P
                nc.vector.tensor_tensor(ex, ex, mask_t[:, j0 : j0 + QN], OP.mult)
                nc.tensor.matmul(
                    ps_att, lhsT=vh[:, ks, :], rhs=ex,
                    start=(ks == 0), stop=(ks == NKS - 1),
                )
                if ks == 0:
                    nc.vector.tensor_copy(out=den_acc, in_=ex)
                else:
                    nc.vector.tensor_tensor(den_acc, den_acc, ex, OP.add)
            ps_den = psum.tile([P, QN], f32, tag="den")
            nc.tensor.matmul(ps_den, lhsT=ones_t, rhs=den_acc, start=True, stop=True)
            rec = small_pool.tile([P, QN], f32, tag="small")
            nc.vector.reciprocal(rec, ps_den)
            nc.vector.tensor_tensor(attn_outT[:, h, :], ps_att, rec, OP.mult)

        # ---------- Phase C: O projection + residual -------------------------
        yT = big_pool.tile([P, DS, QN], f32, tag="big")
        nc.sync.dma_start(yT, xTq_r)
        for mc in range(DS):
            wot = w_pool.tile([P, H, P], f32, tag="w")
            nc.sync.dma_start(wot, wo[mc])
            ps_o = psum.tile([P, QN], f32, tag="proj")
            for hs in range(H):
                nc.tensor.matmul(
                    ps_o, lhsT=wot[:, hs, :], rhs=attn_outT[:, hs, :],
                    start=(hs == 0), stop=(hs == H - 1),
                )
            nc.vector.tensor_tensor(yT[:, mc, :], yT[:, mc, :], ps_o, OP.add)

        # ---------- Phase D: RMSNorm2 + SwiGLU MLP ---------------------------
        h2T = big_pool.tile([P, DS, QN], f32, tag="big")
        rmsnorm(yT, h2T, QN)

        for f0, f1 in F_GROUPS:
            nf = f1 - f0
            hid = hid_pool.tile([P, 11, QN], f32, tag="hid")
            for j in range(nf):
                ffc = f0 + j
                wgt = w_pool.tile([P, DS, P], f32, tag="w")
                nc.sync.dma_start(wgt, wg[ffc])
                ps_g = psum.tile([P, QN], f32, tag="proj")
                for i in range(DS):
                    nc.tensor.matmul(
                        ps_g, lhsT=wgt[:, i, :], rhs=h2T[:, i, :],
                        start=(i == 0), stop=(i == DS - 1),
                    )
                sg = sq_pool.tile([P, QN], f32, tag="sq")
                nc.scalar.activation(sg, ps_g, AF.Silu)
                wut = w_pool.tile([P, DS, P], f32, tag="w")
                nc.sync.dma_start(wut, wu[ffc])
                ps_u = psum.tile([P, QN], f32, tag="proj")
                for i in range(DS):
                    nc.tensor.matmul(
                        ps_u, lhsT=wut[:, i, :], rhs=h2T[:, i, :],
                        start=(i == 0), stop=(i == DS - 1),
                    )
                nc.vector.tensor_tensor(hid[:, j, :], ps_u, sg, OP.mult)
            for mc in range(DS):
                wdt = w_pool.tile([P, 11, P], f32, tag="w")
                nc.sync.dma_start(wdt[:, :nf, :], wd[mc][:, f0:f1, :])
                ps_d = psum.tile([P, QN], f32, tag="score")
                for j in range(nf):
                    nc.tensor.matmul(
                        ps_d, lhsT=wdt[:, j, :], rhs=hid[:, j, :],
                        start=(j == 0), stop=(j == nf - 1),
                    )
                nc.vector.tensor_tensor(yT[:, mc, :], yT[:, mc, :], ps_d, OP.add)

        # ---------- Phase E: transpose to row layout + store ------------------
        for qs in range(QN // P):
            rows = w_pool.tile([P, DS, P], f32, tag="w")
            for mc in range(DS):
                ps_tr = psum.tile([P, P], f32, tag="den")
                nc.tensor.transpose(ps_tr, yT[:, mc, qs * P : (qs + 1) * P], ident)
                nc.vector.tensor_copy(out=rows[:, mc, :], in_=ps_tr)
            nc.sync.dma_start(out_rows[qs * P : (qs + 1) * P, :], rows)

    nc.compile()
    return nc


def _pack_lhsT(w):
    """[M, K] row-major -> lhsT tile layout:
    out[mc, p, ks, c] = w[mc*128 + c, ks*128 + p]."""
    M, K = w.shape
    w4 = w.reshape(M // P, P, K // P, P)  # [mc, c, ks, p]
    return np.ascontiguousarray(w4.transpose(0, 3, 2, 1))


def _prep_inputs(inputs):
    x = np.asarray(inputs["x"], np.float32)
    cos = np.asarray(inputs["cos"], np.float32)
    sin = np.asarray(inputs["sin"], np.float32)
    g1 = np.asarray(inputs["g1"], np.float32)
    g2 = np.asarray(inputs["g2"], np.float32)

    wq = np.asarray(inputs["wq"], np.float32) * g1[None, :]
    wk = np.asarray(inputs["wk"], np.float32) * g1[None, :]
    wv = np.asarray(inputs["wv"], np.float32) * g1[None, :]
    wo = np.asarray(inputs["wo"], np.float32)
    wgate = np.asarray(inputs["w_gate"], np.float32) * g2[None, :]
    wup = np.asarray(inputs["w_up"], np.float32) * g2[None, :]
    wdown = np.asarray(inputs["w_down"], np.float32)

    shared = {
        "wq_pk": _pack_lhsT(wq),
        "wk_pk": _pack_lhsT(wk),
        "wv_pk": _pack_lhsT(wv),
        "wo_pk": _pack_lhsT(wo),
        "wg_pk": _pack_lhsT(wgate),
        "wu_pk": _pack_lhsT(wup),
        "wd_pk": _pack_lhsT(wdown),
    }
    cosT = np.ascontiguousarray(cos.T)                      # [128, S]
    sinT = sin.T.copy()
    sinT[0:64, :] *= -1.0                                   # pre-signed rotate_half
    sinT = np.ascontiguousarray(sinT)
    shared["cos_k"] = cosT
    shared["sin_k"] = sinT

    xT_b = [np.ascontiguousarray(x[b].T) for b in range(B)]  # [D, S]
    scale = 1.0 / np.sqrt(np.float32(HD))

    in_maps = []
    for c in range(NCORES):
        b, qi = c // 4, c % 4
        q0 = qi * QN
        j = np.arange(MEXT)
        m_ext = (np.arange(P)[:, None] <= (q0 + j - (S - P))[None, :]).astype(np.float32)
        in_maps.append(
            dict(
                shared,
                xT=xT_b[b],
                xTq=np.ascontiguousarray(xT_b[b][:, q0 : q0 + QN]),
                cos_q=np.ascontiguousarray(cosT[:, q0 : q0 + QN] * scale),
                sin_q=np.ascontiguousarray(sinT[:, q0 : q0 + QN] * scale),
                mask_ext=np.ascontiguousarray(m_ext),
            )
        )
    return in_maps


def kernel(**inputs):
    global _prog
    from concourse.bass_utils import run_bass_kernel_spmd

    if _prog is None:
        _prog = _build()
    in_maps = _prep_inputs(inputs)
    res = run_bass_kernel_spmd(_prog, in_maps, list(range(NCORES)))
    out = np.empty((B, S, D), np.float32)
    for c in range(NCORES):
        out[c // 4, (c % 4) * QN : (c % 4 + 1) * QN, :] = res.results[c]["out_rows"]
    return out



# revision 9
# speedup vs baseline: 9.7644x; 9.7644x over previous
"""Trainium2 Bass kernel for a dense transformer block (RMSNorm + GQA attention
with RoPE + SwiGLU MLP), tensor-parallel over 8 NeuronCores.

Megatron-style TP=8 with sequence-parallel collectives: core c owns heads
{2c, 2c+1}, KV head c, and FF rows [c*768, (c+1)*768) of an FF dim padded
5504->6144. Each core receives only its 512-token slice of x (f32) plus its
1/8 weight shard (bf16), so the host->device transfer per call is ~8x smaller
than a replicated-weights layout. Device collectives stitch the block back
together: AllGather the rmsnorm'd activations (bf16) before QKV and before
the MLP, ReduceScatter the o-proj and down-proj partial sums (f32) so each
core finishes with exactly its 512 output rows.

Matmuls run in bf16 (f32 PSUM accumulation); the residual stream stays f32.
Softmax runs without max-subtraction (scores sigma~0.8; exp cannot overflow).
Causality is exploited: query chunk qc only visits key subtiles 0..4*(qc+1)-1
of its batch, with a mask only on the 4 diagonal subtiles.
"""

import sys

sys.path.insert(0, "/opt/trn_rl_repo")

import numpy as np
import ml_dtypes

B, S, D = 2, 2048, 2048
H, KVH, HD = 16, 8, 128
FF = 5504
P = 128
DS = D // P          # 16 subtiles of D
T = B * S            # 4096 tokens
TN = 512             # tokens per core shard
NT = T // TN         # 8 token chunks == n cores
HC = H // 8          # 2 heads per core
FFP = 768            # padded FF rows per core (6144 total)
FC = FFP // P        # 6 FF subtiles per core
EPS = 1e-5
NCORES = 8

_prog = None


def _build():
    from contextlib import ExitStack

    import concourse.bass as bass  # noqa: F401
    import concourse.tile as tile
    from concourse import bacc, mybir
    from concourse.masks import make_identity

    f32 = mybir.dt.float32
    bf16 = mybir.dt.bfloat16
    AF = mybir.ActivationFunctionType
    OP = mybir.AluOpType
    GRP = [list(range(NCORES))]

    nc = bacc.Bacc("TRN2", target_bir_lowering=False, debug=False)

    x_sh = nc.dram_tensor("x_sh", [D, TN], f32, kind="ExternalInput").ap()
    wq = nc.dram_tensor("wq_pk", [HC, P, DS, P], bf16, kind="ExternalInput").ap()
    wk = nc.dram_tensor("wk_pk", [P, DS, P], bf16, kind="ExternalInput").ap()
    wv = nc.dram_tensor("wv_pk", [P, DS, P], bf16, kind="ExternalInput").ap()
    wo = nc.dram_tensor("wo_pk", [DS, P, HC, P], bf16, kind="ExternalInput").ap()
    wg = nc.dram_tensor("wg_pk", [FC, P, DS, P], bf16, kind="ExternalInput").ap()
    wu = nc.dram_tensor("wu_pk", [FC, P, DS, P], bf16, kind="ExternalInput").ap()
    wd = nc.dram_tensor("wd_pk", [DS, P, FC, P], bf16, kind="ExternalInput").ap()
    cosk = nc.dram_tensor("cos_k", [P, S], f32, kind="ExternalInput").ap()
    sink = nc.dram_tensor("sin_k", [P, S], f32, kind="ExternalInput").ap()
    maskd = nc.dram_tensor("mask_d", [P, 4, TN], bf16, kind="ExternalInput").ap()
    out_rows = nc.dram_tensor("out_rows", [TN, D], f32, kind="ExternalOutput").ap()

    # Collective buffers (inputs Local, outputs Shared).
    hsh_d = nc.dram_tensor("hsh_d", [D, TN], bf16).ap()
    hcat_d = nc.dram_tensor("hcat_d", [NT, D, TN], bf16, addr_space="Shared").ap()
    ypart_d = nc.dram_tensor("ypart_d", [NT, D, TN], f32).ap()
    ysh_d = nc.dram_tensor("ysh_d", [D, TN], f32).ap()
    h2sh_d = nc.dram_tensor("h2sh_d", [D, TN], bf16).ap()
    h2cat_d = nc.dram_tensor("h2cat_d", [NT, D, TN], bf16, addr_space="Shared").ap()
    mpart_d = nc.dram_tensor("mpart_d", [NT, D, TN], f32).ap()
    msh_d = nc.dram_tensor("msh_d", [D, TN], f32).ap()

    x_r = x_sh.rearrange("(ds p) t -> p ds t", p=P)
    hsh_r = hsh_d.rearrange("(ds p) t -> p ds t", p=P)
    hcat_r = hcat_d.rearrange("n (ds p) t -> p n ds t", p=P)
    ypart_r = ypart_d.rearrange("n (ds p) t -> p n ds t", p=P)
    ysh_r = ysh_d.rearrange("(ds p) t -> p ds t", p=P)
    h2sh_r = h2sh_d.rearrange("(ds p) t -> p ds t", p=P)
    h2cat_r = h2cat_d.rearrange("n (ds p) t -> p n ds t", p=P)
    mpart_r = mpart_d.rearrange("n (ds p) t -> p n ds t", p=P)
    msh_r = msh_d.rearrange("(ds p) t -> p ds t", p=P)

    with tile.TileContext(nc) as tc, ExitStack() as ctx:
        const_pool = ctx.enter_context(tc.tile_pool(name="const", bufs=1))
        big_pool = ctx.enter_context(tc.tile_pool(name="big", bufs=1))     # 32KB
        h_pool = ctx.enter_context(tc.tile_pool(name="h", bufs=2))         # 32KB
        q_pool = ctx.enter_context(tc.tile_pool(name="q", bufs=1))         # 8KB
        kv_pool = ctx.enter_context(tc.tile_pool(name="kv", bufs=1))       # 8KB
        att_pool = ctx.enter_context(tc.tile_pool(name="att", bufs=1))     # 8KB
        w_pool = ctx.enter_context(tc.tile_pool(name="w", bufs=3))         # 12KB
        tab_pool = ctx.enter_context(tc.tile_pool(name="tab", bufs=1))     # 10KB
        stage_pool = ctx.enter_context(tc.tile_pool(name="stage", bufs=3)) # 6KB
        sq_pool = ctx.enter_context(tc.tile_pool(name="sq", bufs=2))       # 4KB
        small_pool = ctx.enter_context(tc.tile_pool(name="small", bufs=3)) # 6KB
        ex_pool = ctx.enter_context(tc.tile_pool(name="ex", bufs=2))       # 2KB
        rope_pool = ctx.enter_context(tc.tile_pool(name="rope", bufs=3))   # 3KB
        psum = ctx.enter_context(tc.tile_pool(name="ps", bufs=2, space="PSUM"))

        ones_f = const_pool.tile([P, P], f32, tag="onesf")
        nc.vector.memset(ones_f, 1.0)
        ones_b = const_pool.tile([P, P], bf16, tag="onesb")
        nc.vector.memset(ones_b, 1.0)
        ident_f = const_pool.tile([P, P], f32, tag="identf")
        make_identity(nc, ident_f)
        ident_b = const_pool.tile([P, P], bf16, tag="identb")
        make_identity(nc, ident_b)
        eps_t = const_pool.tile([P, 1], f32, tag="eps")
        nc.vector.memset(eps_t, EPS)

        cos_t = tab_pool.tile([P, S], f32, tag="cos")
        nc.sync.dma_start(cos_t, cosk)
        sin_t = tab_pool.tile([P, S], f32, tag="sin")
        nc.sync.dma_start(sin_t, sink)
        mask_t = tab_pool.tile([P, 4, TN], bf16, tag="mask")
        nc.sync.dma_start(mask_t, maskd)

        def rmsnorm_cast(xt, dst_bf):
            """dst_bf[:, i, :] (bf16) = rms-normalized xt[:, i, :] (f32)."""
            ps_ss = psum.tile([P, TN], f32, tag="proj")
            for i in range(DS):
                sq = sq_pool.tile([P, TN], f32, tag="sq")
                nc.vector.tensor_tensor(sq, xt[:, i, :], xt[:, i, :], OP.mult)
                nc.tensor.matmul(
                    ps_ss, lhsT=ones_f, rhs=sq, start=(i == 0), stop=(i == DS - 1)
                )
            sqv = small_pool.tile([P, TN], f32, tag="small")
            nc.scalar.activation(sqv, ps_ss, AF.Sqrt, bias=eps_t, scale=1.0 / D)
            rstd = small_pool.tile([P, TN], f32, tag="small")
            nc.vector.reciprocal(rstd, sqv)
            for i in range(DS):
                nc.vector.tensor_tensor(dst_bf[:, i, :], xt[:, i, :], rstd, OP.mult)

        def rope_bf(ps_in, pos, dst):
            """dst (bf16) = ps_in*cos + rotate_half(ps_in)*sin  (sin pre-signed).

            ps_in is the f32 PSUM projection; PSUM+SBUF operand mixes are
            exempt from the equal-base-partition rule, so the rotate-half
            cross-partition reads go straight from PSUM."""
            c_sl = cos_t[:, pos : pos + TN]
            s_sl = sin_t[:, pos : pos + TN]
            a = rope_pool.tile([P, TN], bf16, tag="rope")
            nc.vector.tensor_tensor(a, ps_in, c_sl, OP.mult)
            b = rope_pool.tile([P, TN], bf16, tag="rope")
            nc.vector.tensor_tensor(b[0:64, :], ps_in[64:128, :], s_sl[0:64, :], OP.mult)
            nc.vector.tensor_tensor(b[64:128, :], ps_in[0:64, :], s_sl[64:128, :], OP.mult)
            nc.vector.tensor_tensor(dst, a, b, OP.add)

        # ---------- Phase 0: rmsnorm own shard, AllGather ---------------------
        xt = big_pool.tile([P, DS, TN], f32, tag="big")
        nc.sync.dma_start(xt, x_r)
        hb = h_pool.tile([P, DS, TN], bf16, tag="h")
        rmsnorm_cast(xt, hb)
        nc.sync.dma_start(hsh_r, hb)
        nc.gpsimd.collective_compute(
            "AllGather", mybir.AluOpType.bypass, replica_groups=GRP,
            ins=[hsh_d], outs=[hcat_d],
        )

        # ---------- Phases 1-3 per batch -------------------------------------
        for b in range(B):
            # Phase 1: QKV projections + RoPE for this batch's 2048 tokens.
            qT = q_pool.tile([P, HC, S], bf16, tag="qT")
            kT = kv_pool.tile([P, S], bf16, tag="kT")
            vT = kv_pool.tile([P, S // P, P], bf16, tag="vT")
            for tcl in range(4):
                tcg = 4 * b + tcl
                pos = tcl * TN
                hc_t = h_pool.tile([P, DS, TN], bf16, tag="h")
                nc.sync.dma_start(hc_t, hcat_r[:, tcg, :, :])

                for h in range(HC):
                    wqt = w_pool.tile([P, DS, P], bf16, tag="w")
                    nc.sync.dma_start(wqt, wq[h])
                    ps_q = psum.tile([P, TN], f32, tag="proj")
                    for i in range(DS):
                        nc.tensor.matmul(
                            ps_q, lhsT=wqt[:, i, :], rhs=hc_t[:, i, :],
                            start=(i == 0), stop=(i == DS - 1),
                        )
                    rope_bf(ps_q, pos, qT[:, h, pos : pos + TN])

                wkt = w_pool.tile([P, DS, P], bf16, tag="w")
                nc.sync.dma_start(wkt, wk)
                ps_k = psum.tile([P, TN], f32, tag="proj")
                for i in range(DS):
                    nc.tensor.matmul(
                        ps_k, lhsT=wkt[:, i, :], rhs=hc_t[:, i, :],
                        start=(i == 0), stop=(i == DS - 1),
                    )
                rope_bf(ps_k, pos, kT[:, pos : pos + TN])

                wvt = w_pool.tile([P, DS, P], bf16, tag="w")
                nc.sync.dma_start(wvt, wv)
                ps_v = psum.tile([P, TN], f32, tag="proj")
                for i in range(DS):
                    nc.tensor.matmul(
                        ps_v, lhsT=wvt[:, i, :], rhs=hc_t[:, i, :],
                        start=(i == 0), stop=(i == DS - 1),
                    )
                vts = stage_pool.tile([P, TN], bf16, tag="stb")
                nc.scalar.copy(vts, ps_v)
                for t in range(4):
                    ps_tr = psum.tile([P, P], bf16, tag="den")
                    nc.tensor.transpose(ps_tr, vts[:, t * P : (t + 1) * P], ident_b)
                    nc.vector.tensor_copy(out=vT[:, tcl * 4 + t, :], in_=ps_tr)

            # Phase 2: attention for this batch (2 heads x 4 query chunks).
            attT = att_pool.tile([P, HC, S], bf16, tag="attT")
            for h in range(HC):
                for qcl in range(4):
                    q0 = qcl * TN
                    nks = (qcl + 1) * 4          # visible key subtiles
                    ps_att = psum.tile([P, TN], f32, tag="att")
                    ps_den = psum.tile([P, TN], f32, tag="score")
                    for ks in range(nks):
                        ps_s = psum.tile([P, TN], f32, tag="proj")
                        nc.tensor.matmul(
                            ps_s, lhsT=kT[:, ks * P : (ks + 1) * P],
                            rhs=qT[:, h, q0 : q0 + TN],
                            start=True, stop=True,
                        )
                        ex = ex_pool.tile([P, TN], bf16, tag="ex")
                        nc.scalar.activation(ex, ps_s, AF.Exp)
                        if ks >= nks - 4:
                            nc.vector.tensor_tensor(
                                ex, ex, mask_t[:, ks - (nks - 4), :], OP.mult
                            )
                        nc.tensor.matmul(
                            ps_att, lhsT=vT[:, ks, :], rhs=ex,
                            start=(ks == 0), stop=(ks == nks - 1),
                        )
                        nc.tensor.matmul(
                            ps_den, lhsT=ones_b, rhs=ex,
                            start=(ks == 0), stop=(ks == nks - 1),
                        )
                    rec = small_pool.tile([P, TN], f32, tag="small")
                    nc.vector.reciprocal(rec, ps_den)
                    nc.vector.tensor_tensor(
                        attT[:, h, q0 : q0 + TN], ps_att, rec, OP.mult
                    )

            # Phase 3: o-proj partial sums for this batch's 4 token chunks.
            for mc in range(DS):
                wot = w_pool.tile([P, HC, P], bf16, tag="w")
                nc.sync.dma_start(wot, wo[mc])
                for tcl in range(4):
                    tcg = 4 * b + tcl
                    ps_o = psum.tile([P, TN], f32, tag="att")
                    for h in range(HC):
                        nc.tensor.matmul(
                            ps_o, lhsT=wot[:, h, :],
                            rhs=attT[:, h, tcl * TN : (tcl + 1) * TN],
                            start=(h == 0), stop=(h == HC - 1),
                        )
                    st = stage_pool.tile([P, TN], f32, tag="stf")
                    nc.scalar.copy(st, ps_o)
                    nc.sync.dma_start(ypart_r[:, tcg, mc, :], st)

        nc.gpsimd.collective_compute(
            "ReduceScatter", mybir.AluOpType.add, replica_groups=GRP,
            ins=[ypart_d], outs=[ysh_d],
        )

        # ---------- Phase 4: residual + rmsnorm2 + AllGather ------------------
        yt = big_pool.tile([P, DS, TN], f32, tag="big")
        nc.sync.dma_start(yt, ysh_r)
        for i in range(DS):
            xs = stage_pool.tile([P, TN], f32, tag="stf")
            nc.sync.dma_start(xs, x_r[:, i, :])
            nc.vector.tensor_tensor(yt[:, i, :], yt[:, i, :], xs, OP.add)
        h2b = h_pool.tile([P, DS, TN], bf16, tag="h")
        rmsnorm_cast(yt, h2b)
        nc.sync.dma_start(h2sh_r, h2b)
        nc.gpsimd.collective_compute(
            "AllGather", mybir.AluOpType.bypass, replica_groups=GRP,
            ins=[h2sh_d], outs=[h2cat_d],
        )

        # ---------- Phase 5: SwiGLU MLP (own FF shard, all tokens) -----------
        for tcg in range(NT):
            hc_t = h_pool.tile([P, DS, TN], bf16, tag="h")
            nc.sync.dma_start(hc_t, h2cat_r[:, tcg, :, :])
            hid = att_pool.tile([P, FC, TN], bf16, tag="hid")
            for j in range(FC):
                wgt = w_pool.tile([P, DS, P], bf16, tag="w")
                nc.sync.dma_start(wgt, wg[j])
                ps_g = psum.tile([P, TN], f32, tag="score")
                for i in range(DS):
                    nc.tensor.matmul(
                        ps_g, lhsT=wgt[:, i, :], rhs=hc_t[:, i, :],
                        start=(i == 0), stop=(i == DS - 1),
                    )
                sg = stage_pool.tile([P, TN], bf16, tag="stb")
                nc.scalar.activation(sg, ps_g, AF.Silu)
                wut = w_pool.tile([P, DS, P], bf16, tag="w")
                nc.sync.dma_start(wut, wu[j])
                ps_u = psum.tile([P, TN], f32, tag="att")
                for i in range(DS):
                    nc.tensor.matmul(
                        ps_u, lhsT=wut[:, i, :], rhs=hc_t[:, i, :],
                        start=(i == 0), stop=(i == DS - 1),
                    )
                us = stage_pool.tile([P, TN], bf16, tag="stb")
                nc.scalar.copy(us, ps_u)
                nc.vector.tensor_tensor(hid[:, j, :], us, sg, OP.mult)
            for mc in range(DS):
                wdt = w_pool.tile([P, FC, P], bf16, tag="w")
                nc.sync.dma_start(wdt, wd[mc])
                ps_d = psum.tile([P, TN], f32, tag="proj")
                for j in range(FC):
                    nc.tensor.matmul(
                        ps_d, lhsT=wdt[:, j, :], rhs=hid[:, j, :],
                        start=(j == 0), stop=(j == FC - 1),
                    )
                st = stage_pool.tile([P, TN], f32, tag="stf")
                nc.scalar.copy(st, ps_d)
                nc.sync.dma_start(mpart_r[:, tcg, mc, :], st)

        nc.gpsimd.collective_compute(
            "ReduceScatter", mybir.AluOpType.add, replica_groups=GRP,
            ins=[mpart_d], outs=[msh_d],
        )

        # ---------- Phase 6: final residual + transpose + store ---------------
        for i in range(DS):
            ms = stage_pool.tile([P, TN], f32, tag="stf")
            nc.sync.dma_start(ms, msh_r[:, i, :])
            nc.vector.tensor_tensor(yt[:, i, :], yt[:, i, :], ms, OP.add)
        for qs in range(TN // P):
            rows = h_pool.tile([P, DS, P], f32, tag="rows")
            for mc in range(DS):
                ps_tr = psum.tile([P, P], f32, tag="den")
                nc.tensor.transpose(ps_tr, yt[:, mc, qs * P : (qs + 1) * P], ident_f)
                nc.vector.tensor_copy(out=rows[:, mc, :], in_=ps_tr)
            nc.sync.dma_start(out_rows[qs * P : (qs + 1) * P, :], rows)

    nc.compile()
    return nc


def _pack_lhsT(w):
    """[M, K] row-major -> lhsT tile layout:
    out[mc, p, ks, c] = w[mc*128 + c, ks*128 + p]."""
    M, K = w.shape
    w4 = w.reshape(M // P, P, K // P, P)  # [mc, c, ks, p]
    return np.ascontiguousarray(w4.transpose(0, 3, 2, 1))


def _prep_inputs(inputs):
    bf = ml_dtypes.bfloat16
    x = np.asarray(inputs["x"], np.float32)
    cos = np.asarray(inputs["cos"], np.float32)
    sin = np.asarray(inputs["sin"], np.float32)
    g1 = np.asarray(inputs["g1"], np.float32)
    g2 = np.asarray(inputs["g2"], np.float32)

    scale = 1.0 / np.sqrt(np.float32(HD))
    wq_f = np.asarray(inputs["wq"], np.float32) * g1[None, :] * scale
    wk_f = np.asarray(inputs["wk"], np.float32) * g1[None, :]
    wv_f = np.asarray(inputs["wv"], np.float32) * g1[None, :]
    wo_f = np.asarray(inputs["wo"], np.float32)
    wg_f = np.asarray(inputs["w_gate"], np.float32) * g2[None, :]
    wu_f = np.asarray(inputs["w_up"], np.float32) * g2[None, :]
    wd_f = np.asarray(inputs["w_down"], np.float32)

    wg_pad = np.zeros((NCORES * FFP, D), np.float32)
    wg_pad[:FF] = wg_f
    wu_pad = np.zeros((NCORES * FFP, D), np.float32)
    wu_pad[:FF] = wu_f
    wd_pad = np.zeros((D, NCORES * FFP), np.float32)
    wd_pad[:, :FF] = wd_f

    cosT = np.ascontiguousarray(cos.T)                       # [128, S]
    sinT = sin.T.copy()
    sinT[0:64, :] *= -1.0                                    # pre-signed rotate_half
    sinT = np.ascontiguousarray(sinT)

    j = np.arange(TN)
    ksl = np.arange(4 * P).reshape(4, P)
    mask_d = (ksl[:, :, None] <= j[None, None, :]).astype(np.float32)  # [4,P,TN]
    mask_d = np.ascontiguousarray(mask_d.transpose(1, 0, 2)).astype(bf)

    xT_b = [np.ascontiguousarray(x[b].T) for b in range(B)]  # [D, S]

    in_maps = []
    for c in range(NCORES):
        b, qi = c // 4, c % 4
        wq_c = wq_f[2 * c * HD : (2 * c + HC) * HD]
        wk_c = wk_f[c * HD : (c + 1) * HD]
        wv_c = wv_f[c * HD : (c + 1) * HD]
        wo_c = wo_f[:, 2 * c * HD : (2 * c + HC) * HD]
        wg_c = wg_pad[c * FFP : (c + 1) * FFP]
        wu_c = wu_pad[c * FFP : (c + 1) * FFP]
        wd_c = wd_pad[:, c * FFP : (c + 1) * FFP]
        in_maps.append(
            dict(
                x_sh=np.ascontiguousarray(xT_b[b][:, qi * TN : (qi + 1) * TN]),
                wq_pk=_pack_lhsT(wq_c).astype(bf),
                wk_pk=_pack_lhsT(wk_c)[0].astype(bf),
                wv_pk=_pack_lhsT(wv_c)[0].astype(bf),
                wo_pk=_pack_lhsT(wo_c).astype(bf),
                wg_pk=_pack_lhsT(wg_c).astype(bf),
                wu_pk=_pack_lhsT(wu_c).astype(bf),
                wd_pk=_pack_lhsT(wd_c).astype(bf),
                cos_k=cosT,
                sin_k=sinT,
                mask_d=mask_d,
            )
        )
    return in_maps


def kernel(**inputs):
    global _prog
    from concourse.bass_utils import run_bass_kernel_spmd

    if _prog is None:
        _prog = _build()
    in_maps = _prep_inputs(inputs)
    res = run_bass_kernel_spmd(_prog, in_maps, list(range(NCORES)))
    out = np.empty((B, S, D), np.float32)
    for c in range(NCORES):
        out[c // 4, (c % 4) * TN : (c % 4 + 1) * TN, :] = res.results[c]["out_rows"]
    return out


# revision 28
# speedup vs baseline: 12.7392x; 1.3046x over previous
"""Trainium2 Bass kernel for a dense transformer block (RMSNorm + GQA attention
with RoPE + SwiGLU MLP), tensor-parallel over 8 NeuronCores.

Megatron-style TP=8 with sequence-parallel collectives: core c owns heads
{2c, 2c+1}, KV head c, and FF rows [c*768, (c+1)*768) of an FF dim padded
5504->6144. Each core receives only its 512-token slice of x (f32) plus its
1/8 weight shard (bf16), so the host->device transfer per call is ~8x smaller
than a replicated-weights layout. Device collectives stitch the block back
together: AllGather the rmsnorm'd activations (bf16) before QKV and before
the MLP, ReduceScatter the o-proj and down-proj partial sums (f32) so each
core finishes with exactly its 512 output rows.

Matmuls run in bf16 (f32 PSUM accumulation); the residual stream stays f32.
Softmax runs without max-subtraction (scores sigma~0.8; exp cannot overflow).
Causality is exploited: query chunk qc only visits key subtiles 0..4*(qc+1)-1
of its batch, with a mask only on the 4 diagonal subtiles.
"""

import sys

sys.path.insert(0, "/opt/trn_rl_repo")

import numpy as np
import ml_dtypes

B, S, D = 2, 2048, 2048
H, KVH, HD = 16, 8, 128
FF = 5504
P = 128
DS = D // P          # 16 subtiles of D
T = B * S            # 4096 tokens
TN = 512             # tokens per core shard
NT = T // TN         # 8 token chunks == n cores
HC = H // 8          # 2 heads per core
FFP = 768            # padded FF rows per core (6144 total)
FC = FFP // P        # 6 FF subtiles per core
EPS = 1e-5
NCORES = 8

_prog = None


def _build():
    from contextlib import ExitStack

    import concourse.bass as bass  # noqa: F401
    import concourse.tile as tile
    from concourse import bacc, mybir
    from concourse.masks import make_identity

    f32 = mybir.dt.float32
    bf16 = mybir.dt.bfloat16
    AF = mybir.ActivationFunctionType
    OP = mybir.AluOpType
    GRP = [list(range(NCORES))]

    nc = bacc.Bacc("TRN2", target_bir_lowering=False, debug=False)

    x_sh = nc.dram_tensor("x_sh", [D, TN], bf16, kind="ExternalInput").ap()
    wq = nc.dram_tensor("wq_pk", [HC, P, DS, P], bf16, kind="ExternalInput").ap()
    wk = nc.dram_tensor("wk_pk", [P, DS, P], bf16, kind="ExternalInput").ap()
    wv = nc.dram_tensor("wv_pk", [P, DS, P], bf16, kind="ExternalInput").ap()
    wo = nc.dram_tensor("wo_pk", [DS, P, HC, P], bf16, kind="ExternalInput").ap()
    wg = nc.dram_tensor("wg_pk", [FC, P, DS, P], bf16, kind="ExternalInput").ap()
    wu = nc.dram_tensor("wu_pk", [FC, P, DS, P], bf16, kind="ExternalInput").ap()
    wd = nc.dram_tensor("wd_pk", [DS, P, FC, P], bf16, kind="ExternalInput").ap()
    cosk = nc.dram_tensor("cos_k", [P, S], bf16, kind="ExternalInput").ap()
    sink = nc.dram_tensor("sin_k", [P, S], bf16, kind="ExternalInput").ap()
    # out_rows carries only the attn+mlp contribution (bf16); the host adds
    # the f32 residual x back, so x is never rounded on the output path.
    out_rows = nc.dram_tensor("out_rows", [TN, D], bf16, kind="ExternalOutput").ap()

    # Collective buffers (inputs Local, outputs Shared).
    hsh_d = nc.dram_tensor("hsh_d", [D, TN], bf16).ap()
    hcat_d = nc.dram_tensor("hcat_d", [NT, D, TN], bf16, addr_space="Shared").ap()
    ypart_d = nc.dram_tensor("ypart_d", [NT, D, TN], f32).ap()
    ysh_d = nc.dram_tensor("ysh_d", [D, TN], f32).ap()
    h2sh_d = nc.dram_tensor("h2sh_d", [D, TN], bf16).ap()
    h2cat_d = nc.dram_tensor("h2cat_d", [NT, D, TN], bf16, addr_space="Shared").ap()
    mpart_d = nc.dram_tensor("mpart_d", [NT, D, TN], f32).ap()
    msh_d = nc.dram_tensor("msh_d", [D, TN], f32).ap()

    x_r = x_sh.rearrange("(ds p) t -> p ds t", p=P)
    hsh_r = hsh_d.rearrange("(ds p) t -> p ds t", p=P)
    hcat_r = hcat_d.rearrange("n (ds p) t -> p n ds t", p=P)
    ypart_r = ypart_d.rearrange("n (ds p) t -> p n ds t", p=P)
    ysh_r = ysh_d.rearrange("(ds p) t -> p ds t", p=P)
    h2sh_r = h2sh_d.rearrange("(ds p) t -> p ds t", p=P)
    h2cat_r = h2cat_d.rearrange("n (ds p) t -> p n ds t", p=P)
    mpart_r = mpart_d.rearrange("n (ds p) t -> p n ds t", p=P)
    msh_r = msh_d.rearrange("(ds p) t -> p ds t", p=P)

    with tile.TileContext(nc) as tc, ExitStack() as ctx:
        const_pool = ctx.enter_context(tc.tile_pool(name="const", bufs=1))
        big_pool = ctx.enter_context(tc.tile_pool(name="big", bufs=1))     # 32KB
        h_pool = ctx.enter_context(tc.tile_pool(name="h", bufs=2))         # 32KB
        q_pool = ctx.enter_context(tc.tile_pool(name="q", bufs=1))         # 8KB
        kv_pool = ctx.enter_context(tc.tile_pool(name="kv", bufs=1))       # 8KB
        att_pool = ctx.enter_context(tc.tile_pool(name="att", bufs=1))     # 8KB
        w_pool = ctx.enter_context(tc.tile_pool(name="w", bufs=3))         # 12KB
        tab_pool = ctx.enter_context(tc.tile_pool(name="tab", bufs=1))     # 10KB
        stage_pool = ctx.enter_context(tc.tile_pool(name="stage", bufs=3)) # 6KB
        sq_pool = ctx.enter_context(tc.tile_pool(name="sq", bufs=2))       # 4KB
        small_pool = ctx.enter_context(tc.tile_pool(name="small", bufs=3)) # 6KB
        ex_pool = ctx.enter_context(tc.tile_pool(name="ex", bufs=2))       # 2KB
        rope_pool = ctx.enter_context(tc.tile_pool(name="rope", bufs=3))   # 3KB
        rows_pool = ctx.enter_context(tc.tile_pool(name="rows", bufs=2))   # 8KB
        psum = ctx.enter_context(tc.tile_pool(name="ps", bufs=2, space="PSUM"))

        ones_f = const_pool.tile([P, P], f32, tag="onesf")
        nc.vector.memset(ones_f, 1.0)
        ones_b = const_pool.tile([P, P], bf16, tag="onesb")
        nc.vector.memset(ones_b, 1.0)
        ident_f = const_pool.tile([P, P], f32, tag="identf")
        make_identity(nc, ident_f)
        ident_b = const_pool.tile([P, P], bf16, tag="identb")
        make_identity(nc, ident_b)
        eps_t = const_pool.tile([P, 1], f32, tag="eps")
        nc.vector.memset(eps_t, EPS)

        # cos/sin ship as bf16 and are upcast once; rope multiplies them with
        # f32 PSUM operands, which requires matching f32 dtype on the DVE.
        cosb = w_pool.tile([P, S], bf16, tag="w")
        nc.sync.dma_start(cosb, cosk)
        cos_t = tab_pool.tile([P, S], f32, tag="cos")
        nc.vector.tensor_copy(out=cos_t, in_=cosb)
        sinb = w_pool.tile([P, S], bf16, tag="w")
        nc.sync.dma_start(sinb, sink)
        sin_t = tab_pool.tile([P, S], f32, tag="sin")
        nc.vector.tensor_copy(out=sin_t, in_=sinb)
        # Causal mask for the 4 diagonal key subtiles of a 512-query chunk:
        # mask[p, ks, j] = 1 if ks*128 + p <= j else 0.
        mask_t = tab_pool.tile([P, 4, TN], bf16, tag="mask")
        nc.gpsimd.memset(mask_t, 1.0)
        for ksl in range(4):
            nc.gpsimd.affine_select(
                out=mask_t[:, ksl, :], in_=mask_t[:, ksl, :],
                pattern=[[1, TN]], compare_op=OP.is_ge,
                fill=0.0, base=-128 * ksl, channel_multiplier=-1,
            )

        def rmsnorm_cast(xt, dst_bf):
            """dst_bf[:, i, :] (bf16) = rms-normalized xt[:, i, :] (f32)."""
            ps_ss = psum.tile([P, TN], f32, tag="proj")
            for i in range(DS):
                sq = sq_pool.tile([P, TN], f32, tag="sq")
                nc.vector.tensor_tensor(sq, xt[:, i, :], xt[:, i, :], OP.mult)
                nc.tensor.matmul(
                    ps_ss, lhsT=ones_f, rhs=sq, start=(i == 0), stop=(i == DS - 1)
                )
            sqv = small_pool.tile([P, TN], f32, tag="small")
            nc.scalar.activation(sqv, ps_ss, AF.Sqrt, bias=eps_t, scale=1.0 / D)
            rstd = small_pool.tile([P, TN], f32, tag="small")
            nc.vector.reciprocal(rstd, sqv)
            for i in range(DS):
                nc.vector.tensor_tensor(dst_bf[:, i, :], xt[:, i, :], rstd, OP.mult)

        def rope_bf(ps_in, pos, dst):
            """dst (bf16) = ps_in*cos + rotate_half(ps_in)*sin  (sin pre-signed).

            ps_in is the f32 PSUM projection; PSUM+SBUF operand mixes are
            exempt from the equal-base-partition rule, so the rotate-half
            cross-partition reads go straight from PSUM."""
            c_sl = cos_t[:, pos : pos + TN]
            s_sl = sin_t[:, pos : pos + TN]
            a = rope_pool.tile([P, TN], bf16, tag="rope")
            nc.vector.tensor_tensor(a, ps_in, c_sl, OP.mult)
            b = rope_pool.tile([P, TN], bf16, tag="rope")
            nc.vector.tensor_tensor(b[0:64, :], ps_in[64:128, :], s_sl[0:64, :], OP.mult)
            nc.vector.tensor_tensor(b[64:128, :], ps_in[0:64, :], s_sl[64:128, :], OP.mult)
            nc.vector.tensor_tensor(dst, a, b, OP.add)

        # ---------- Phase 0: rmsnorm own shard, AllGather ---------------------
        # x ships bf16 (the host re-adds the f32 x on the output path, so the
        # device copy only feeds the two rmsnorms); upcast once so downstream
        # DVE ops keep same-dtype operands.
        xb = h_pool.tile([P, DS, TN], bf16, tag="h")
        nc.sync.dma_start(xb, x_r)
        xt = big_pool.tile([P, DS, TN], f32, tag="big")
        nc.vector.tensor_copy(out=xt, in_=xb)
        hb = h_pool.tile([P, DS, TN], bf16, tag="h")
        rmsnorm_cast(xt, hb)
        nc.sync.dma_start(hsh_r, hb)
        nc.gpsimd.collective_compute(
            "AllGather", mybir.AluOpType.bypass, replica_groups=GRP,
            ins=[hsh_d], outs=[hcat_d],
        )

        # ---------- Phases 1-3 per batch -------------------------------------
        for b in range(B):
            # Phase 1: QKV projections + RoPE for this batch's 2048 tokens.
            qT = q_pool.tile([P, HC, S], bf16, tag="qT")
            kT = kv_pool.tile([P, S], bf16, tag="kT")
            vT = kv_pool.tile([P, S // P, P], bf16, tag="vT")
            for tcl in range(4):
                tcg = 4 * b + tcl
                pos = tcl * TN
                hc_t = h_pool.tile([P, DS, TN], bf16, tag="h")
                nc.sync.dma_start(hc_t, hcat_r[:, tcg, :, :])

                for h in range(HC):
                    wqt = w_pool.tile([P, DS, P], bf16, tag="w")
                    nc.sync.dma_start(wqt, wq[h])
                    ps_q = psum.tile([P, TN], f32, tag="proj")
                    for i in range(DS):
                        nc.tensor.matmul(
                            ps_q, lhsT=wqt[:, i, :], rhs=hc_t[:, i, :],
                            start=(i == 0), stop=(i == DS - 1),
                        )
                    rope_bf(ps_q, pos, qT[:, h, pos : pos + TN])

                wkt = w_pool.tile([P, DS, P], bf16, tag="w")
                nc.sync.dma_start(wkt, wk)
                ps_k = psum.tile([P, TN], f32, tag="proj")
                for i in range(DS):
                    nc.tensor.matmul(
                        ps_k, lhsT=wkt[:, i, :], rhs=hc_t[:, i, :],
                        start=(i == 0), stop=(i == DS - 1),
                    )
                rope_bf(ps_k, pos, kT[:, pos : pos + TN])

                wvt = w_pool.tile([P, DS, P], bf16, tag="w")
                nc.sync.dma_start(wvt, wv)
                ps_v = psum.tile([P, TN], f32, tag="proj")
                for i in range(DS):
                    nc.tensor.matmul(
                        ps_v, lhsT=wvt[:, i, :], rhs=hc_t[:, i, :],
                        start=(i == 0), stop=(i == DS - 1),
                    )
                vts = stage_pool.tile([P, TN], bf16, tag="stb")
                nc.scalar.copy(vts, ps_v)
                for t in range(4):
                    ps_tr = psum.tile([P, P], bf16, tag="den")
                    nc.tensor.transpose(ps_tr, vts[:, t * P : (t + 1) * P], ident_b)
                    nc.vector.tensor_copy(out=vT[:, tcl * 4 + t, :], in_=ps_tr)

            # Phase 2: attention for this batch (2 heads x 4 query chunks).
            attT = att_pool.tile([P, HC, S], bf16, tag="attT")
            for h in range(HC):
                for qcl in range(4):
                    q0 = qcl * TN
                    nks = (qcl + 1) * 4          # visible key subtiles
                    ps_att = psum.tile([P, TN], f32, tag="att")
                    ps_den = psum.tile([P, TN], f32, tag="score")
                    for ks in range(nks):
                        ps_s = psum.tile([P, TN], f32, tag="proj")
                        nc.tensor.matmul(
                            ps_s, lhsT=kT[:, ks * P : (ks + 1) * P],
                            rhs=qT[:, h, q0 : q0 + TN],
                            start=True, stop=True,
                        )
                        ex = ex_pool.tile([P, TN], bf16, tag="ex")
                        nc.scalar.activation(ex, ps_s, AF.Exp)
                        if ks >= nks - 4:
                            nc.vector.tensor_tensor(
                                ex, ex, mask_t[:, ks - (nks - 4), :], OP.mult
                            )
                        nc.tensor.matmul(
                            ps_att, lhsT=vT[:, ks, :], rhs=ex,
                            start=(ks == 0), stop=(ks == nks - 1),
                        )
                        nc.tensor.matmul(
                            ps_den, lhsT=ones_b, rhs=ex,
                            start=(ks == 0), stop=(ks == nks - 1),
                        )
                    rec = small_pool.tile([P, TN], f32, tag="small")
                    nc.vector.reciprocal(rec, ps_den)
                    nc.vector.tensor_tensor(
                        attT[:, h, q0 : q0 + TN], ps_att, rec, OP.mult
                    )

            # Phase 3: o-proj partial sums for this batch's 4 token chunks.
            for mc in range(DS):
                wot = w_pool.tile([P, HC, P], bf16, tag="w")
                nc.sync.dma_start(wot, wo[mc])
                for tcl in range(4):
                    tcg = 4 * b + tcl
                    ps_o = psum.tile([P, TN], f32, tag="att")
                    for h in range(HC):
                        nc.tensor.matmul(
                            ps_o, lhsT=wot[:, h, :],
                            rhs=attT[:, h, tcl * TN : (tcl + 1) * TN],
                            start=(h == 0), stop=(h == HC - 1),
                        )
                    st = stage_pool.tile([P, TN], f32, tag="stf")
                    nc.scalar.copy(st, ps_o)
                    nc.sync.dma_start(ypart_r[:, tcg, mc, :], st)

        nc.gpsimd.collective_compute(
            "ReduceScatter", mybir.AluOpType.add, replica_groups=GRP,
            ins=[ypart_d], outs=[ysh_d],
        )

        # ---------- Phase 4: residual + rmsnorm2 + AllGather ------------------
        yt = big_pool.tile([P, DS, TN], f32, tag="big")
        for i in range(DS):
            ys = stage_pool.tile([P, TN], f32, tag="stf")
            nc.sync.dma_start(ys, ysh_r[:, i, :])
            xb2 = stage_pool.tile([P, TN], bf16, tag="stb")
            nc.sync.dma_start(xb2, x_r[:, i, :])
            xs = stage_pool.tile([P, TN], f32, tag="stf")
            nc.vector.tensor_copy(out=xs, in_=xb2)
            nc.vector.tensor_tensor(yt[:, i, :], ys, xs, OP.add)
        h2b = h_pool.tile([P, DS, TN], bf16, tag="h")
        rmsnorm_cast(yt, h2b)
        nc.sync.dma_start(h2sh_r, h2b)
        nc.gpsimd.collective_compute(
            "AllGather", mybir.AluOpType.bypass, replica_groups=GRP,
            ins=[h2sh_d], outs=[h2cat_d],
        )

        # ---------- Phase 5: SwiGLU MLP (own FF shard, all tokens) -----------
        for tcg in range(NT):
            hc_t = h_pool.tile([P, DS, TN], bf16, tag="h")
            nc.sync.dma_start(hc_t, h2cat_r[:, tcg, :, :])
            hid = att_pool.tile([P, FC, TN], bf16, tag="hid")
            for j in range(FC):
                wgt = w_pool.tile([P, DS, P], bf16, tag="w")
                nc.sync.dma_start(wgt, wg[j])
                ps_g = psum.tile([P, TN], f32, tag="score")
                for i in range(DS):
                    nc.tensor.matmul(
                        ps_g, lhsT=wgt[:, i, :], rhs=hc_t[:, i, :],
                        start=(i == 0), stop=(i == DS - 1),
                    )
                sg = stage_pool.tile([P, TN], bf16, tag="stb")
                nc.scalar.activation(sg, ps_g, AF.Silu)
                wut = w_pool.tile([P, DS, P], bf16, tag="w")
                nc.sync.dma_start(wut, wu[j])
                ps_u = psum.tile([P, TN], f32, tag="att")
                for i in range(DS):
                    nc.tensor.matmul(
                        ps_u, lhsT=wut[:, i, :], rhs=hc_t[:, i, :],
                        start=(i == 0), stop=(i == DS - 1),
                    )
                us = stage_pool.tile([P, TN], bf16, tag="stb")
                nc.scalar.copy(us, ps_u)
                nc.vector.tensor_tensor(hid[:, j, :], us, sg, OP.mult)
            for mc in range(DS):
                wdt = w_pool.tile([P, FC, P], bf16, tag="w")
                nc.sync.dma_start(wdt, wd[mc])
                ps_d = psum.tile([P, TN], f32, tag="proj")
                for j in range(FC):
                    nc.tensor.matmul(
                        ps_d, lhsT=wdt[:, j, :], rhs=hid[:, j, :],
                        start=(j == 0), stop=(j == FC - 1),
                    )
                st = stage_pool.tile([P, TN], f32, tag="stf")
                nc.scalar.copy(st, ps_d)
                nc.sync.dma_start(mpart_r[:, tcg, mc, :], st)

        nc.gpsimd.collective_compute(
            "ReduceScatter", mybir.AluOpType.add, replica_groups=GRP,
            ins=[mpart_d], outs=[msh_d],
        )

        # ---------- Phase 6: contribution = attn + mlp, transpose, store ------
        ctt = h_pool.tile([P, DS, TN], bf16, tag="h")
        for mc in range(DS):
            ys = stage_pool.tile([P, TN], f32, tag="stf")
            nc.sync.dma_start(ys, ysh_r[:, mc, :])
            ms = stage_pool.tile([P, TN], f32, tag="stf")
            nc.sync.dma_start(ms, msh_r[:, mc, :])
            nc.vector.tensor_tensor(ctt[:, mc, :], ys, ms, OP.add)
        for qs in range(TN // P):
            rows = rows_pool.tile([P, DS, P], bf16, tag="rows")
            for mc in range(DS):
                ps_tr = psum.tile([P, P], bf16, tag="den")
                nc.tensor.transpose(ps_tr, ctt[:, mc, qs * P : (qs + 1) * P], ident_b)
                nc.vector.tensor_copy(out=rows[:, mc, :], in_=ps_tr)
            nc.sync.dma_start(out_rows[qs * P : (qs + 1) * P, :], rows)

    nc.compile()
    return nc


def _pack_lhsT(w):
    """[M, K] row-major -> lhsT tile layout:
    out[mc, p, ks, c] = w[mc*128 + c, ks*128 + p]."""
    M, K = w.shape
    w4 = w.reshape(M // P, P, K // P, P)  # [mc, c, ks, p]
    return np.ascontiguousarray(w4.transpose(0, 3, 2, 1))


def _prep_inputs(inputs):
    bf = ml_dtypes.bfloat16
    x = np.asarray(inputs["x"], np.float32)
    cos = np.asarray(inputs["cos"], np.float32)
    sin = np.asarray(inputs["sin"], np.float32)
    g1 = np.asarray(inputs["g1"], np.float32)
    g2 = np.asarray(inputs["g2"], np.float32)

    scale = 1.0 / np.sqrt(np.float32(HD))
    wq_f = np.asarray(inputs["wq"], np.float32) * g1[None, :] * scale
    wk_f = np.asarray(inputs["wk"], np.float32) * g1[None, :]
    wv_f = np.asarray(inputs["wv"], np.float32) * g1[None, :]
    wo_f = np.asarray(inputs["wo"], np.float32)
    wg_f = np.asarray(inputs["w_gate"], np.float32) * g2[None, :]
    wu_f = np.asarray(inputs["w_up"], np.float32) * g2[None, :]
    wd_f = np.asarray(inputs["w_down"], np.float32)

    wg_pad = np.zeros((NCORES * FFP, D), np.float32)
    wg_pad[:FF] = wg_f
    wu_pad = np.zeros((NCORES * FFP, D), np.float32)
    wu_pad[:FF] = wu_f
    wd_pad = np.zeros((D, NCORES * FFP), np.float32)
    wd_pad[:, :FF] = wd_f

    cosT = np.ascontiguousarray(cos.T).astype(bf)            # [128, S]
    sinT = sin.T.copy()
    sinT[0:64, :] *= -1.0                                    # pre-signed rotate_half
    sinT = np.ascontiguousarray(sinT).astype(bf)

    xT_b = [np.ascontiguousarray(x[b].T) for b in range(B)]  # [D, S]

    in_maps = []
    for c in range(NCORES):
        b, qi = c // 4, c % 4
        wq_c = wq_f[2 * c * HD : (2 * c + HC) * HD]
        wk_c = wk_f[c * HD : (c + 1) * HD]
        wv_c = wv_f[c * HD : (c + 1) * HD]
        wo_c = wo_f[:, 2 * c * HD : (2 * c + HC) * HD]
        wg_c = wg_pad[c * FFP : (c + 1) * FFP]
        wu_c = wu_pad[c * FFP : (c + 1) * FFP]
        wd_c = wd_pad[:, c * FFP : (c + 1) * FFP]
        in_maps.append(
            dict(
                x_sh=np.ascontiguousarray(xT_b[b][:, qi * TN : (qi + 1) * TN]).astype(bf),
                wq_pk=_pack_lhsT(wq_c).astype(bf),
                wk_pk=_pack_lhsT(wk_c)[0].astype(bf),
                wv_pk=_pack_lhsT(wv_c)[0].astype(bf),
                wo_pk=_pack_lhsT(wo_c).astype(bf),
                wg_pk=_pack_lhsT(wg_c).astype(bf),
                wu_pk=_pack_lhsT(wu_c).astype(bf),
                wd_pk=_pack_lhsT(wd_c).astype(bf),
                cos_k=cosT,
                sin_k=sinT,
            )
        )
    return in_maps


def kernel(**inputs):
    global _prog
    from concourse.bass_utils import run_bass_kernel_spmd

    if _prog is None:
        _prog = _build()
    in_maps = _prep_inputs(inputs)
    res = run_bass_kernel_spmd(_prog, in_maps, list(range(NCORES)))
    x = np.asarray(inputs["x"], np.float32)
    out = np.empty((B, S, D), np.float32)
    for c in range(NCORES):
        b, q0 = c // 4, (c % 4) * TN
        out[b, q0 : q0 + TN, :] = x[b, q0 : q0 + TN, :] + res.results[c][
            "out_rows"
        ].astype(np.float32)
    return out
